# revision 23
# baseline (speedup 1.0000x reference)
"""Causal multi-head attention block (qkv proj + attention + out proj) on 8
Trainium2 NeuronCores.

Sharding: core c = 2*b + hg handles batch b (of 4) and head-group hg (8 of 16
heads).  Each core computes qkv for its heads, causal attention, and a partial
out-projection (its 512 rows of w_out); the host sums the two head-group
partials per batch.

v2 design (vs fp32r baseline):
  - x is pre-transposed and pre-cast to bf16 on the HOST (xT [DIM, T]); all
    PE operands are bf16 (PSUM accumulation stays fp32), which removes the
    128 PE transposes + their DVE evacuations, removes the fp32r narrow-
    matmul 4x penalty on causal-diagonal tiles, and enables fast weight load.
  - weights are pre-cast to bf16 on the host (halves weight DMA bytes).
  - scores are computed transposed, S^T[k, q], two heads per pair via PE row
    tiling (K=64 at tile (0,0)/(64,0)); the attention k-loop runs in chunks
    of 2 k-tiles: [4 score MMs in 64x128 mode][2 batched exp ACTs][4 PV MMs
    in 128x128 mode], so PE tiling-mode switches happen per chunk instead of
    per matmul and fillers (qkv/out_proj units) are pumped only at chunk
    boundaries.
  - V tiles carry a ones column per head (V_aug) so P @ V_aug accumulates
    numerator and softmax denominator together (softmax runs without
    max-subtraction: scores ~ N(0,1), exp safe).
  - normalization is SBUF-only: DVE reciprocal on the [2, 512] denominator
    rows, partition-broadcast DMA of the reciprocals, fused multiply+cast
    into the bf16 out-proj stationary tiles.
  - input DMAs ride the ACT hardware queue (idle early), output/aux DMAs the
    SP queue; x tiles are loaded before weights so the PE starts early.
"""

import sys

if "/opt/trn_rl_repo" not in sys.path:
    sys.path.insert(0, "/opt/trn_rl_repo")

import numpy as np
import ml_dtypes

import concourse.bass as bass
import concourse.mybir as mybir
import concourse.tile as tile
from concourse import bacc
from concourse.bass_utils import run_bass_kernel_spmd

DIM = 1024
N_HEAD = 16
HD = 64
B, T = 4, 2048
HG = 8          # heads per core
CQ = HG * HD    # 512 feature columns per group
NCORES = 8
NT = T // 128   # 16 t-subtiles
NQ = T // 512   # 4 quarters / q-blocks

f32 = mybir.dt.float32
bf16 = mybir.dt.bfloat16
Exp = mybir.ActivationFunctionType.Exp
AluAdd = mybir.AluOpType.add


def build_nc():
    nc = bacc.Bacc(None, target_bir_lowering=False)
    xt_d = nc.declare_dram_parameter("xT", [128, 8 * T], bf16, isOutput=False)
    wqk_d = nc.declare_dram_parameter("wqk", [128, 8 * 2 * CQ], bf16,
                                      isOutput=False)
    wv_d = nc.declare_dram_parameter("wv", [128, 8 * CQ], bf16, isOutput=False)
    wo_d = nc.declare_dram_parameter("wo", [128, 4 * DIM], bf16, isOutput=False)
    mv_d = nc.declare_dram_parameter("maskv", [128, NT], f32, isOutput=False)
    out_d = nc.declare_dram_parameter("out", [T, DIM], f32, isOutput=True)

    with tile.TileContext(nc) as tc:
        with tc.tile_pool(name="pp", bufs=1) as pp, \
             tc.tile_pool(name="qtp", bufs=2) as qtp, \
             tc.tile_pool(name="p_p", bufs=4) as p_p, \
             tc.tile_pool(name="at_p", bufs=3) as at_p, \
             tc.tile_pool(name="den_p", bufs=2) as den_p, \
             tc.tile_pool(name="bcs_p", bufs=3) as bcs_p, \
             tc.tile_pool(name="out_p", bufs=3) as out_p, \
             tc.tile_pool(name="dram_p", bufs=3, space="DRAM") as dram_p, \
             tc.tile_pool(name="ps_aux", bufs=2, space="PSUM") as ps_aux, \
             tc.tile_pool(name="ps_s", bufs=2, space="PSUM") as ps_s, \
             tc.tile_pool(name="ps_pv", bufs=1, space="PSUM") as ps_pv:

            # ---- persistent tensors, loaded fine-grained so the first
            # qk unit only waits for quarter-0 x slices + the m=0/1 weight
            # columns instead of the full 8.5 MB input set.  x slices ride
            # the SP queue, weights the ACT queue, so they stream in
            # parallel.
            # SBUF x image is quarter-major (q, kb, 512) and the qk/wqk
            # image is column-group-major (mc, kb, 256); the host writes the
            # SAME layouts to DRAM, so every input load is one fully
            # contiguous DMA.
            xts_all = pp.tile([128, 8 * T], bf16, name="xts", tag="xts")

            def xslice(kb, c0, c1):
                q = c0 // 512
                base = (q * 8 + kb) * 512
                return xts_all[:, base + (c0 - q * 512):base + (c1 - q * 512)]

            wqk_all = pp.tile([128, 8 * 2 * CQ], bf16, name="wqka", tag="wqka")

            def wqkslice(kb, m):
                mc, mr = m // 2, m % 2
                base = (mc * 8 + kb) * 256 + mr * 128
                return wqk_all[:, base:base + 128]

            wv_all = pp.tile([128, 8 * CQ], bf16, name="wva", tag="wva")
            wv_sb = [wv_all[:, k * CQ:(k + 1) * CQ] for k in range(8)]
            wo_all = pp.tile([128, 4 * DIM], bf16, name="woa", tag="woa")
            wo_sb = [wo_all[:, m * DIM:(m + 1) * DIM] for m in range(4)]
            mv_sb = pp.tile([128, NT], f32, name="maskv_sb", tag="maskv_sb")

            for h in range(4):
                nc.sync.dma_start(out=xts_all[:, h * 1024:(h + 1) * 1024],
                                  in_=xt_d[:, h * 1024:(h + 1) * 1024])
            for h in range(2):
                nc.scalar.dma_start(out=wqk_all[:, h * 1024:(h + 1) * 1024],
                                    in_=wqk_d[:, h * 1024:(h + 1) * 1024])
            nc.scalar.dma_start(out=wqk_all[:, 2048:8192],
                                in_=wqk_d[:, 2048:8192])
            nc.scalar.dma_start(out=mv_sb, in_=mv_d[:, :])
            nc.scalar.dma_start(out=wv_all, in_=wv_d[:, :])
            for q in range(1, 4):
                nc.sync.dma_start(out=xts_all[:, q * 4096:(q + 1) * 4096],
                                  in_=xt_d[:, q * 4096:(q + 1) * 4096])
            nc.scalar.dma_start(out=wo_all, in_=wo_d[:, :])

            # ---- constants ----
            # one 128x128 causal strip: keep where q_local >= k_local
            dstrip = pp.tile([128, 128], bf16, name="dstrip", tag="dstrip")
            nc.gpsimd.memset(dstrip, 1.0)
            nc.gpsimd.affine_select(
                out=dstrip, in_=dstrip, compare_op=mybir.AluOpType.is_ge,
                fill=0.0, base=0, pattern=[[1, 128]], channel_multiplier=-1)
            onescol = pp.tile([128, HG], bf16, name="onescol", tag="onescol")
            nc.vector.memset(onescol, 1.0)

            # ---- persistent attention tensors ----
            kt = [pp.tile([128, T], bf16, name=f"kt{m}", tag=f"kt{m}") for m in range(4)]
            vaug = [pp.tile([128, HG * 65], bf16, name=f"vaug{t}", tag=f"vaug{t}")
                    for t in range(NT)]

            qt_cur = {}    # quarter -> [4 pair tiles [128, 512]]
            ats_cur = {}   # qb -> [4 pair tiles [128, 512]]

            # ---------- qkv quarter units ----------
            def qkv_units(q):
                units = []
                qt_cur[q] = [None] * 4

                def qk_unit(m):
                    pq = ps_aux.tile([128, 512], f32, name="mm", tag="aux")
                    for kb in range(8):
                        nc.tensor.matmul(
                            pq, wqkslice(kb, m),
                            xslice(kb, q * 512, (q + 1) * 512),
                            start=(kb == 0), stop=(kb == 7)).annotate("mm:qk")
                    if m < 4:
                        qtile = qtp.tile([128, 512], bf16, name=f"qt{m}", tag=f"qt{m}")
                        nc.vector.tensor_copy(qtile, pq)
                        qt_cur[q][m] = qtile
                    else:
                        nc.vector.tensor_copy(
                            kt[m - 4][:, q * 512:(q + 1) * 512], pq)
                for m in range(8):
                    units.append(lambda m=m: qk_unit(m))

                def v_unit(ti):
                    pv = ps_aux.tile([128, 512], f32, name="mm", tag="aux")
                    t0 = (q * 4 + ti) * 128
                    for kb in range(8):
                        nc.tensor.matmul(
                            pv, xslice(kb, t0, t0 + 128), wv_sb[kb],
                            start=(kb == 0), stop=(kb == 7)).annotate("mm:v")
                    vt = vaug[q * 4 + ti]
                    vt3 = vt.rearrange("p (h w) -> p h w", w=65)
                    nc.vector.tensor_copy(
                        vt3[:, :, 0:64], pv.rearrange("p (h w) -> p h w", w=64))
                    nc.vector.tensor_copy(
                        vt3[:, :, 64:65], onescol.rearrange("p (h w) -> p h w", w=1))
                    nc.vector.tensor_scalar_mul(
                        vt, vt, mv_sb[:, (q * 4 + ti):(q * 4 + ti + 1)])
                for ti in range(4):
                    units.append(lambda ti=ti: v_unit(ti))
                return units

            # ---------- out_proj units for one q-block ----------
            def outproj_units(qb):
                units = []

                def op_unit(ti, nb):
                    ats = ats_cur[qb]
                    po = ps_aux.tile([128, 512], f32, name="mm", tag="aux")
                    for m in range(4):
                        nc.tensor.matmul(
                            po, ats[m][:, ti * 128:(ti + 1) * 128],
                            wo_sb[m][:, nb * 512:(nb + 1) * 512],
                            start=(m == 0), stop=(m == 3)).annotate("mm:op")
                    ob = out_p.tile([128, 512], f32, name="ob", tag="ob")
                    nc.vector.tensor_copy(ob, po)
                    t0 = (qb * 4 + ti) * 128
                    # out-writes ride the ACT queue so the latency-critical
                    # normalization chains never queue behind them on SP
                    nc.scalar.dma_start(
                        out=out_d[t0:t0 + 128, nb * 512:(nb + 1) * 512], in_=ob)
                for ti in range(4):
                    for nb in range(2):
                        units.append(lambda ti=ti, nb=nb: op_unit(ti, nb))
                return units

            # ---------- attention pair: chunked k-loop ----------
            def att_pair(qb, m, pump):
                nk = 4 * (qb + 1)
                pvp = ps_pv.tile([65, 1024], f32, name="pv", tag="pv")
                qt = qt_cur[qb][m]

                for c0 in range(0, nk, 2):
                    ktis = [k for k in (c0, c0 + 1) if k < nk]
                    # --- score matmuls: 64x128 mode run ---
                    sps = []
                    for kti in ktis:
                        j = kti - 4 * qb
                        w0 = 128 * j if j > 0 else 0
                        sp = ps_s.tile([128, 1024], f32, name="s", tag="s")
                        nc.tensor.matmul(
                            sp[:, w0:512],
                            kt[m][0:64, kti * 128:(kti + 1) * 128],
                            qt[0:64, w0:512], start=True, stop=True
                            ).annotate("mm:s1")
                        nc.tensor.matmul(
                            sp[:, 512 + w0:1024],
                            kt[m][64:128, kti * 128:(kti + 1) * 128],
                            qt[64:128, w0:512], start=True, stop=True
                            ).annotate("mm:s2")
                        sps.append((kti, sp, w0, j))
                    # --- exp (ACT) + causal strip (DVE) ---
                    ppts = []
                    for kti, sp, w0, j in sps:
                        ppt = p_p.tile([128, 1024], bf16, name="p", tag="p")
                        p3 = ppt.rearrange("p (h w) -> p h w", w=512)
                        s3 = sp.rearrange("p (h w) -> p h w", w=512)
                        if j < 0:
                            nc.scalar.activation(p3, s3, Exp, scale=0.125)
                        else:
                            nc.scalar.activation(
                                p3[:, :, w0:512], s3[:, :, w0:512], Exp,
                                scale=0.125)
                            for h in range(2):
                                nc.vector.tensor_mul(
                                    ppt[:, h * 512 + w0:h * 512 + w0 + 128],
                                    ppt[:, h * 512 + w0:h * 512 + w0 + 128],
                                    dstrip)
                        ppts.append((kti, ppt, w0))
                    # pump fillers here: their 128-mode MMs sit ahead of
                    # the PV matmuls in the PE queue and execute while the
                    # exp ACTs run (PV's input dependency)
                    pump()
                    # --- PV matmuls: 128x128 mode run ---
                    for kti, ppt, w0 in ppts:
                        stop = (kti == nk - 1)
                        nc.tensor.matmul(
                            pvp[:, w0:512],
                            vaug[kti][:, (2 * m) * 65:(2 * m + 1) * 65],
                            ppt[:, w0:512], start=(kti == 0), stop=stop
                            ).annotate("mm:pv1")
                        nc.tensor.matmul(
                            pvp[:, 512 + w0:1024],
                            vaug[kti][:, (2 * m + 1) * 65:(2 * m + 2) * 65],
                            ppt[:, 512 + w0:1024], start=(kti == 0), stop=stop
                            ).annotate("mm:pv2")

                # --- normalization ---
                # evacuate pvp fast (um + dn) so the next pair's PV can start,
                # then run the reciprocal/broadcast chain off the critical path:
                # dn row -> DRAM -> [128,8] so the DVE reciprocal runs 8
                # elems/lane, -> DRAM -> partition-broadcast load -> multiply.
                dn = den_p.tile([1, 1024], f32, name="dn", tag="dn")
                nc.vector.tensor_copy(dn, pvp[64:65, :])
                um = at_p.tile([128, 512], bf16, name=f"um{m}", tag=f"um{m}")
                for h in range(2):
                    nc.vector.tensor_copy(
                        um[h * 64:(h + 1) * 64, :],
                        pvp[0:64, h * 512:(h + 1) * 512])
                d1 = dram_p.tile([2, 512], f32, name="d1", tag="d1")
                nc.sync.dma_start(
                    out=d1.rearrange("i w -> (i w)").rearrange("(a b) -> a b", a=1),
                    in_=dn)
                den128 = den_p.tile([128, 8], f32, name="den128", tag="den128")
                nc.sync.dma_start(
                    out=den128,
                    in_=d1.rearrange("i w -> (i w)").rearrange("(p c) -> p c", c=8))
                rec128 = den_p.tile([128, 8], bf16, name="rec128", tag="rec128")
                with nc.allow_low_precision(reason="bf16 softmax denominators"):
                    nc.vector.reciprocal(rec128, den128)
                d2 = dram_p.tile([2, 512], bf16, name="d2", tag="d2")
                nc.sync.dma_start(
                    out=d2.rearrange("i w -> (i w)").rearrange("(p c) -> p c", c=8),
                    in_=rec128)
                bcs = bcs_p.tile([128, 512], bf16, name="bcs", tag="bcs")
                for h in range(2):
                    nc.sync.dma_start(
                        out=bcs[h * 64:(h + 1) * 64, :],
                        in_=d2[h:h + 1, :].partition_broadcast(64))
                atm = at_p.tile([128, 512], bf16, name=f"at{m}", tag=f"at{m}")
                ats_cur[qb][m] = atm
                nc.vector.tensor_mul(atm, um, bcs)

            def run_phase(tasks, fillers, n_units):
                """tasks: closures taking pump(); fillers pumped proportionally."""
                nf = len(fillers)
                state = {"fi": 0, "ai": 0}

                def pump():
                    state["ai"] += 1
                    while state["fi"] * n_units < state["ai"] * nf \
                            and state["fi"] < nf:
                        fillers[state["fi"]]()
                        state["fi"] += 1
                for t in tasks:
                    t(pump)
                while state["fi"] < nf:
                    fillers[state["fi"]]()
                    state["fi"] += 1

            # ---------------- emission schedule ----------------
            for u in qkv_units(0):
                u()
            for qb in range(NQ):
                ats_cur[qb] = [None] * 4

            def phase_tasks(qb):
                def mk(m):
                    def t(pump):
                        att_pair(qb, m, pump)
                    return t
                return [mk(m) for m in range(4)]

            # pumps per phase: sum over pairs of ceil(nk/2)
            # qb0: 4*2=8, qb1: 4*4=16, qb2: 4*6=24, qb3: 4*8=32
            run_phase(phase_tasks(0), qkv_units(1), 8)
            run_phase(phase_tasks(1), qkv_units(2) + outproj_units(0), 16)
            run_phase(phase_tasks(2), qkv_units(3) + outproj_units(1), 24)
            run_phase(phase_tasks(3), outproj_units(2), 32)
            for u in outproj_units(NQ - 1):
                u()
    nc.finalize()
    return nc


_NC_CACHE = {}


def _get_nc():
    if "nc" not in _NC_CACHE:
        _NC_CACHE["nc"] = build_nc()
    return _NC_CACHE["nc"]


def _make_in_maps(x, w_qkv, w_out, attn_mask):
    x = np.asarray(x, dtype=np.float32)
    w_qkv = np.asarray(w_qkv, dtype=np.float32)
    w_out = np.asarray(w_out, dtype=np.float32)
    am = np.asarray(attn_mask)
    bf = ml_dtypes.bfloat16
    in_maps = []
    for c in range(NCORES):
        b, hg = c // 2, c % 2
        xt = x[b].T.astype(bf)                      # [1024, 2048]
        # SBUF image: [128 part, (q, kb, 512)]
        xt_c = np.ascontiguousarray(
            xt.reshape(8, 128, 4, 512).transpose(1, 2, 0, 3).reshape(128, -1))
        wqk = np.concatenate(
            [w_qkv[:, hg * CQ:(hg + 1) * CQ],
             w_qkv[:, DIM + hg * CQ:DIM + (hg + 1) * CQ]], axis=1).astype(bf)
        # SBUF image: [128 part, (mc, kb, 256)]
        wqk_c = np.ascontiguousarray(
            wqk.reshape(8, 128, 4, 256).transpose(1, 2, 0, 3).reshape(128, -1))
        wv = w_qkv[:, 2 * DIM + hg * CQ:2 * DIM + (hg + 1) * CQ].astype(bf)
        wv_c = np.ascontiguousarray(
            wv.reshape(8, 128, CQ).transpose(1, 0, 2).reshape(128, -1))
        wo = w_out[hg * CQ:(hg + 1) * CQ, :].astype(bf)
        wo_c = np.ascontiguousarray(
            wo.reshape(4, 128, DIM).transpose(1, 0, 2).reshape(128, -1))
        mv_c = np.ascontiguousarray(
            am[b].astype(np.float32).reshape(NT, 128).T)
        in_maps.append({
            "xT": xt_c,
            "wqk": wqk_c,
            "wv": wv_c,
            "wo": wo_c,
            "maskv": mv_c,
        })
    return in_maps


def run(x, w_qkv, w_out, attn_mask, trace=False):
    nc = _get_nc()
    in_maps = _make_in_maps(x, w_qkv, w_out, attn_mask)
    res = run_bass_kernel_spmd(nc, in_maps, list(range(NCORES)), trace=trace)
    outs = [res.results[c]["out"] for c in range(NCORES)]
    full = np.stack([outs[2 * b] + outs[2 * b + 1] for b in range(B)], axis=0)
    return full.astype(np.float32), res


def kernel(x, w_qkv, w_out, attn_mask):
    full, _ = run(x, w_qkv, w_out, attn_mask, trace=False)
    return full


# revision 24
# speedup vs baseline: 1.0801x; 1.0801x over previous
"""Causal multi-head attention block (qkv proj + attention + out proj) on 8
Trainium2 NeuronCores.

Sharding: core c = 2*b + hg handles batch b (of 4) and head-group hg (8 of 16
heads).  Each core computes qkv for its heads, causal attention, and a partial
out-projection (its 512 rows of w_out); the host sums the two head-group
partials per batch.

v2 design (vs fp32r baseline):
  - x is pre-transposed and pre-cast to bf16 on the HOST (xT [DIM, T]); all
    PE operands are bf16 (PSUM accumulation stays fp32), which removes the
    128 PE transposes + their DVE evacuations, removes the fp32r narrow-
    matmul 4x penalty on causal-diagonal tiles, and enables fast weight load.
  - weights are pre-cast to bf16 on the host (halves weight DMA bytes).
  - scores are computed transposed, S^T[k, q], two heads per pair via PE row
    tiling (K=64 at tile (0,0)/(64,0)); the attention k-loop runs in chunks
    of 2 k-tiles: [4 score MMs in 64x128 mode][2 batched exp ACTs][4 PV MMs
    in 128x128 mode], so PE tiling-mode switches happen per chunk instead of
    per matmul and fillers (qkv/out_proj units) are pumped only at chunk
    boundaries.
  - V tiles carry a ones column per head (V_aug) so P @ V_aug accumulates
    numerator and softmax denominator together (softmax runs without
    max-subtraction: scores ~ N(0,1), exp safe).
  - normalization is SBUF-only: DVE reciprocal on the [2, 512] denominator
    rows, partition-broadcast DMA of the reciprocals, fused multiply+cast
    into the bf16 out-proj stationary tiles.
  - input DMAs ride the ACT hardware queue (idle early), output/aux DMAs the
    SP queue; x tiles are loaded before weights so the PE starts early.
"""

import sys

if "/opt/trn_rl_repo" not in sys.path:
    sys.path.insert(0, "/opt/trn_rl_repo")

import numpy as np
import ml_dtypes

import concourse.bass as bass
import concourse.mybir as mybir
import concourse.tile as tile
from concourse import bacc
from concourse.bass_utils import run_bass_kernel_spmd

DIM = 1024
N_HEAD = 16
HD = 64
B, T = 4, 2048
HG = 8          # heads per core
CQ = HG * HD    # 512 feature columns per group
NCORES = 8
NT = T // 128   # 16 t-subtiles
NQ = T // 512   # 4 quarters / q-blocks

f32 = mybir.dt.float32
bf16 = mybir.dt.bfloat16
Exp = mybir.ActivationFunctionType.Exp
AluAdd = mybir.AluOpType.add


def build_nc():
    nc = bacc.Bacc(None, target_bir_lowering=False)
    xt_d = nc.declare_dram_parameter("xT", [128, 8 * T], bf16, isOutput=False)
    wqk_d = nc.declare_dram_parameter("wqk", [128, 8 * 2 * CQ], bf16,
                                      isOutput=False)
    wv_d = nc.declare_dram_parameter("wv", [128, 8 * CQ], bf16, isOutput=False)
    wo_d = nc.declare_dram_parameter("wo", [128, 4 * DIM], bf16, isOutput=False)
    mv_d = nc.declare_dram_parameter("maskv", [128, NT], f32, isOutput=False)
    out_d = nc.declare_dram_parameter("out", [T, DIM], f32, isOutput=True)

    with tile.TileContext(nc) as tc:
        with tc.tile_pool(name="pp", bufs=1) as pp, \
             tc.tile_pool(name="qtp", bufs=2) as qtp, \
             tc.tile_pool(name="p_p", bufs=4) as p_p, \
             tc.tile_pool(name="at_p", bufs=3) as at_p, \
             tc.tile_pool(name="den_p", bufs=2) as den_p, \
             tc.tile_pool(name="bcs_p", bufs=3) as bcs_p, \
             tc.tile_pool(name="out_p", bufs=3) as out_p, \
             tc.tile_pool(name="dram_p", bufs=3, space="DRAM") as dram_p, \
             tc.tile_pool(name="ps_aux", bufs=2, space="PSUM") as ps_aux, \
             tc.tile_pool(name="ps_s", bufs=2, space="PSUM") as ps_s, \
             tc.tile_pool(name="ps_pv", bufs=1, space="PSUM") as ps_pv:

            # ---- persistent tensors, loaded fine-grained so the first
            # qk unit only waits for quarter-0 x slices + the m=0/1 weight
            # columns instead of the full 8.5 MB input set.  x slices ride
            # the SP queue, weights the ACT queue, so they stream in
            # parallel.
            # SBUF x image is quarter-major (q, kb, 512) and the qk/wqk
            # image is column-group-major (mc, kb, 256); the host writes the
            # SAME layouts to DRAM, so every input load is one fully
            # contiguous DMA.
            xts_all = pp.tile([128, 8 * T], bf16, name="xts", tag="xts")

            def xslice(kb, c0, c1):
                q = c0 // 512
                base = (q * 8 + kb) * 512
                return xts_all[:, base + (c0 - q * 512):base + (c1 - q * 512)]

            wqk_all = pp.tile([128, 8 * 2 * CQ], bf16, name="wqka", tag="wqka")

            def wqkslice(kb, m):
                mc, mr = m // 2, m % 2
                base = (mc * 8 + kb) * 256 + mr * 128
                return wqk_all[:, base:base + 128]

            wv_all = pp.tile([128, 8 * CQ], bf16, name="wva", tag="wva")
            wv_sb = [wv_all[:, k * CQ:(k + 1) * CQ] for k in range(8)]
            wo_all = pp.tile([128, 4 * DIM], bf16, name="woa", tag="woa")
            wo_sb = [wo_all[:, m * DIM:(m + 1) * DIM] for m in range(4)]
            mv_sb = pp.tile([128, NT], f32, name="maskv_sb", tag="maskv_sb")

            for h in range(4):
                nc.sync.dma_start(out=xts_all[:, h * 1024:(h + 1) * 1024],
                                  in_=xt_d[:, h * 1024:(h + 1) * 1024])
            for h in range(2):
                nc.scalar.dma_start(out=wqk_all[:, h * 1024:(h + 1) * 1024],
                                    in_=wqk_d[:, h * 1024:(h + 1) * 1024])
            nc.scalar.dma_start(out=wqk_all[:, 2048:8192],
                                in_=wqk_d[:, 2048:8192])
            nc.scalar.dma_start(out=mv_sb, in_=mv_d[:, :])
            nc.scalar.dma_start(out=wv_all, in_=wv_d[:, :])
            for q in range(1, 4):
                nc.sync.dma_start(out=xts_all[:, q * 4096:(q + 1) * 4096],
                                  in_=xt_d[:, q * 4096:(q + 1) * 4096])
            nc.scalar.dma_start(out=wo_all, in_=wo_d[:, :])

            # ---- constants ----
            # one 128x128 causal strip: keep where q_local >= k_local
            dstrip = pp.tile([128, 128], bf16, name="dstrip", tag="dstrip")
            nc.gpsimd.memset(dstrip, 1.0)
            nc.gpsimd.affine_select(
                out=dstrip, in_=dstrip, compare_op=mybir.AluOpType.is_ge,
                fill=0.0, base=0, pattern=[[1, 128]], channel_multiplier=-1)
            onescol = pp.tile([128, HG], bf16, name="onescol", tag="onescol")
            nc.vector.memset(onescol, 1.0)

            # ---- persistent attention tensors ----
            kt = [pp.tile([128, T], bf16, name=f"kt{m}", tag=f"kt{m}") for m in range(4)]
            vaug = [pp.tile([128, HG * 65], bf16, name=f"vaug{t}", tag=f"vaug{t}")
                    for t in range(NT)]

            qt_cur = {}    # quarter -> [4 pair tiles [128, 512]]
            ats_cur = {}   # qb -> [4 pair tiles [128, 512]]

            # ---------- qkv quarter units ----------
            def qkv_units(q):
                units = []
                qt_cur[q] = [None] * 4

                def qk_unit(m):
                    pq = ps_aux.tile([128, 512], f32, name="mm", tag="aux")
                    for kb in range(8):
                        nc.tensor.matmul(
                            pq, wqkslice(kb, m),
                            xslice(kb, q * 512, (q + 1) * 512),
                            start=(kb == 0), stop=(kb == 7)).annotate("mm:qk")
                    if m < 4:
                        qtile = qtp.tile([128, 512], bf16, name=f"qt{m}", tag=f"qt{m}")
                        nc.vector.tensor_copy(qtile, pq)
                        qt_cur[q][m] = qtile
                    else:
                        nc.vector.tensor_copy(
                            kt[m - 4][:, q * 512:(q + 1) * 512], pq)
                for m in range(8):
                    units.append(lambda m=m: qk_unit(m))

                def v_unit(ti):
                    pv = ps_aux.tile([128, 512], f32, name="mm", tag="aux")
                    t0 = (q * 4 + ti) * 128
                    for kb in range(8):
                        nc.tensor.matmul(
                            pv, xslice(kb, t0, t0 + 128), wv_sb[kb],
                            start=(kb == 0), stop=(kb == 7)).annotate("mm:v")
                    vt = vaug[q * 4 + ti]
                    vt3 = vt.rearrange("p (h w) -> p h w", w=65)
                    nc.vector.tensor_copy(
                        vt3[:, :, 0:64], pv.rearrange("p (h w) -> p h w", w=64))
                    nc.vector.tensor_copy(
                        vt3[:, :, 64:65], onescol.rearrange("p (h w) -> p h w", w=1))
                    nc.vector.tensor_scalar_mul(
                        vt, vt, mv_sb[:, (q * 4 + ti):(q * 4 + ti + 1)])
                for ti in range(4):
                    units.append(lambda ti=ti: v_unit(ti))
                return units

            # ---------- out_proj units for one q-block ----------
            def outproj_units(qb):
                units = []

                def op_unit(ti, nb):
                    ats = ats_cur[qb]
                    po = ps_aux.tile([128, 512], f32, name="mm", tag="aux")
                    for m in range(4):
                        nc.tensor.matmul(
                            po, ats[m][:, ti * 128:(ti + 1) * 128],
                            wo_sb[m][:, nb * 512:(nb + 1) * 512],
                            start=(m == 0), stop=(m == 3)).annotate("mm:op")
                    ob = out_p.tile([128, 512], f32, name="ob", tag="ob")
                    nc.vector.tensor_copy(ob, po)
                    t0 = (qb * 4 + ti) * 128
                    # out-writes ride the gpsimd software DGE so the
                    # latency-critical normalization chains never queue
                    # behind them on SP
                    nc.gpsimd.dma_start(
                        out=out_d[t0:t0 + 128, nb * 512:(nb + 1) * 512], in_=ob)
                for ti in range(4):
                    for nb in range(2):
                        units.append(lambda ti=ti, nb=nb: op_unit(ti, nb))
                return units

            # ---------- attention pair: chunked k-loop ----------
            def att_pair(qb, m, pump):
                nk = 4 * (qb + 1)
                pvp = ps_pv.tile([65, 1024], f32, name="pv", tag="pv")
                qt = qt_cur[qb][m]

                for c0 in range(0, nk, 2):
                    ktis = [k for k in (c0, c0 + 1) if k < nk]
                    # --- score matmuls: 64x128 mode run ---
                    sps = []
                    for kti in ktis:
                        j = kti - 4 * qb
                        w0 = 128 * j if j > 0 else 0
                        sp = ps_s.tile([128, 1024], f32, name="s", tag="s")
                        nc.tensor.matmul(
                            sp[:, w0:512],
                            kt[m][0:64, kti * 128:(kti + 1) * 128],
                            qt[0:64, w0:512], start=True, stop=True
                            ).annotate("mm:s1")
                        nc.tensor.matmul(
                            sp[:, 512 + w0:1024],
                            kt[m][64:128, kti * 128:(kti + 1) * 128],
                            qt[64:128, w0:512], start=True, stop=True
                            ).annotate("mm:s2")
                        sps.append((kti, sp, w0, j))
                    # --- exp (ACT) + causal strip (DVE) ---
                    ppts = []
                    for kti, sp, w0, j in sps:
                        ppt = p_p.tile([128, 1024], bf16, name="p", tag="p")
                        p3 = ppt.rearrange("p (h w) -> p h w", w=512)
                        s3 = sp.rearrange("p (h w) -> p h w", w=512)
                        if j < 0:
                            nc.scalar.activation(p3, s3, Exp, scale=0.125)
                        else:
                            nc.scalar.activation(
                                p3[:, :, w0:512], s3[:, :, w0:512], Exp,
                                scale=0.125)
                            for h in range(2):
                                nc.vector.tensor_mul(
                                    ppt[:, h * 512 + w0:h * 512 + w0 + 128],
                                    ppt[:, h * 512 + w0:h * 512 + w0 + 128],
                                    dstrip)
                        ppts.append((kti, ppt, w0))
                    # pump fillers here: their 128-mode MMs sit ahead of
                    # the PV matmuls in the PE queue and execute while the
                    # exp ACTs run (PV's input dependency)
                    pump()
                    # --- PV matmuls: 128x128 mode run ---
                    for kti, ppt, w0 in ppts:
                        stop = (kti == nk - 1)
                        nc.tensor.matmul(
                            pvp[:, w0:512],
                            vaug[kti][:, (2 * m) * 65:(2 * m + 1) * 65],
                            ppt[:, w0:512], start=(kti == 0), stop=stop
                            ).annotate("mm:pv1")
                        nc.tensor.matmul(
                            pvp[:, 512 + w0:1024],
                            vaug[kti][:, (2 * m + 1) * 65:(2 * m + 2) * 65],
                            ppt[:, 512 + w0:1024], start=(kti == 0), stop=stop
                            ).annotate("mm:pv2")

                # --- normalization ---
                # evacuate pvp fast (um + dn) so the next pair's PV can start,
                # then run the reciprocal/broadcast chain off the critical path:
                # dn row -> DRAM -> [128,8] so the DVE reciprocal runs 8
                # elems/lane, -> DRAM -> partition-broadcast load -> multiply.
                dn = den_p.tile([1, 1024], f32, name="dn", tag="dn")
                nc.vector.tensor_copy(dn, pvp[64:65, :])
                um = at_p.tile([128, 512], bf16, name=f"um{m}", tag=f"um{m}")
                for h in range(2):
                    nc.vector.tensor_copy(
                        um[h * 64:(h + 1) * 64, :],
                        pvp[0:64, h * 512:(h + 1) * 512])
                d1 = dram_p.tile([2, 512], f32, name="d1", tag="d1")
                nc.sync.dma_start(
                    out=d1.rearrange("i w -> (i w)").rearrange("(a b) -> a b", a=1),
                    in_=dn)
                den128 = den_p.tile([128, 8], f32, name="den128", tag="den128")
                nc.sync.dma_start(
                    out=den128,
                    in_=d1.rearrange("i w -> (i w)").rearrange("(p c) -> p c", c=8))
                rec128 = den_p.tile([128, 8], bf16, name="rec128", tag="rec128")
                with nc.allow_low_precision(reason="bf16 softmax denominators"):
                    nc.vector.reciprocal(rec128, den128)
                d2 = dram_p.tile([2, 512], bf16, name="d2", tag="d2")
                nc.sync.dma_start(
                    out=d2.rearrange("i w -> (i w)").rearrange("(p c) -> p c", c=8),
                    in_=rec128)
                bcs = bcs_p.tile([128, 512], bf16, name="bcs", tag="bcs")
                for h in range(2):
                    nc.sync.dma_start(
                        out=bcs[h * 64:(h + 1) * 64, :],
                        in_=d2[h:h + 1, :].partition_broadcast(64))
                atm = at_p.tile([128, 512], bf16, name=f"at{m}", tag=f"at{m}")
                ats_cur[qb][m] = atm
                nc.vector.tensor_mul(atm, um, bcs)

            def run_phase(tasks, fillers, n_units):
                """tasks: closures taking pump(); fillers pumped proportionally."""
                nf = len(fillers)
                state = {"fi": 0, "ai": 0}

                def pump():
                    state["ai"] += 1
                    while state["fi"] * n_units < state["ai"] * nf \
                            and state["fi"] < nf:
                        fillers[state["fi"]]()
                        state["fi"] += 1
                for t in tasks:
                    t(pump)
                while state["fi"] < nf:
                    fillers[state["fi"]]()
                    state["fi"] += 1

            # ---------------- emission schedule ----------------
            for u in qkv_units(0):
                u()
            for qb in range(NQ):
                ats_cur[qb] = [None] * 4

            def phase_tasks(qb):
                def mk(m):
                    def t(pump):
                        att_pair(qb, m, pump)
                    return t
                return [mk(m) for m in range(4)]

            # pumps per phase: sum over pairs of ceil(nk/2)
            # qb0: 4*2=8, qb1: 4*4=16, qb2: 4*6=24, qb3: 4*8=32
            run_phase(phase_tasks(0), qkv_units(1), 8)
            run_phase(phase_tasks(1), qkv_units(2) + outproj_units(0), 16)
            run_phase(phase_tasks(2), qkv_units(3) + outproj_units(1), 24)
            run_phase(phase_tasks(3), outproj_units(2), 32)
            for u in outproj_units(NQ - 1):
                u()
    nc.finalize()
    return nc


_NC_CACHE = {}


def _get_nc():
    if "nc" not in _NC_CACHE:
        _NC_CACHE["nc"] = build_nc()
    return _NC_CACHE["nc"]


def _make_in_maps(x, w_qkv, w_out, attn_mask):
    x = np.asarray(x, dtype=np.float32)
    w_qkv = np.asarray(w_qkv, dtype=np.float32)
    w_out = np.asarray(w_out, dtype=np.float32)
    am = np.asarray(attn_mask)
    bf = ml_dtypes.bfloat16
    in_maps = []
    for c in range(NCORES):
        b, hg = c // 2, c % 2
        xt = x[b].T.astype(bf)                      # [1024, 2048]
        # SBUF image: [128 part, (q, kb, 512)]
        xt_c = np.ascontiguousarray(
            xt.reshape(8, 128, 4, 512).transpose(1, 2, 0, 3).reshape(128, -1))
        wqk = np.concatenate(
            [w_qkv[:, hg * CQ:(hg + 1) * CQ],
             w_qkv[:, DIM + hg * CQ:DIM + (hg + 1) * CQ]], axis=1).astype(bf)
        # SBUF image: [128 part, (mc, kb, 256)]
        wqk_c = np.ascontiguousarray(
            wqk.reshape(8, 128, 4, 256).transpose(1, 2, 0, 3).reshape(128, -1))
        wv = w_qkv[:, 2 * DIM + hg * CQ:2 * DIM + (hg + 1) * CQ].astype(bf)
        wv_c = np.ascontiguousarray(
            wv.reshape(8, 128, CQ).transpose(1, 0, 2).reshape(128, -1))
        wo = w_out[hg * CQ:(hg + 1) * CQ, :].astype(bf)
        wo_c = np.ascontiguousarray(
            wo.reshape(4, 128, DIM).transpose(1, 0, 2).reshape(128, -1))
        mv_c = np.ascontiguousarray(
            am[b].astype(np.float32).reshape(NT, 128).T)
        in_maps.append({
            "xT": xt_c,
            "wqk": wqk_c,
            "wv": wv_c,
            "wo": wo_c,
            "maskv": mv_c,
        })
    return in_maps


def run(x, w_qkv, w_out, attn_mask, trace=False):
    nc = _get_nc()
    in_maps = _make_in_maps(x, w_qkv, w_out, attn_mask)
    res = run_bass_kernel_spmd(nc, in_maps, list(range(NCORES)), trace=trace)
    outs = [res.results[c]["out"] for c in range(NCORES)]
    full = np.stack([outs[2 * b] + outs[2 * b + 1] for b in range(B)], axis=0)
    return full.astype(np.float32), res


def kernel(x, w_qkv, w_out, attn_mask):
    full, _ = run(x, w_qkv, w_out, attn_mask, trace=False)
    return full
